# revision 19
# baseline (speedup 1.0000x reference)
"""Trainium2 Bass kernel for nn_DeepReservoir (3-layer masked reservoir with
parametric sine activations and input skips).

Strategy (8 NeuronCores, data-parallel over batch):
  - Shard batch (65536) -> 8192 rows/core; replicate small weights.
  - Transposed layout on device: units on partitions, batch on free dim.
    h^T = W^T @ x^T chains across layers with zero on-device transposes.
  - All matmul operands bf16 (same PE rate as f32r, but fast weight load
    and half the SBUF/DMA traffic).  PSUM stays f32.
  - sine(z) = a*sin(f z)*exp(-d|z|) with exp(-d u) ~ c0 + c1 u (deg-1):
      sin via ScalarE ACT, and either
        (v2a) u' = |c1 z| via ScalarE ACT Abs; E = c0 - u' (DVE ts 4x);
              h = E * sin (DVE tt 2x)
        (v2c) one custom DVE op h = (c0 - c1'|z|) * sin straight from PSUM
              (fuses abs + affine + multiply into the single PSUM drain).
  - Skip adds: h = v + s with s in PSUM; a tunable fraction of the s tiles
    is first copied to SBUF bf16 by ScalarE so the DVE add runs at 2x —
    balances ScalarE vs DVE.
  - Output stored bf16 and upconverted on host (halves the 48 MiB/core
    output stream).
  - Layer chain software-pipelined across batch chunks: PE emission order is
    L0(0), then L1(c), L0(c+1), L2(c).
"""

import numpy as np
import ml_dtypes

import concourse.bacc as bacc
import concourse.mybir as mybir
from concourse.tile import TileContext
from concourse import bass_utils

AF = mybir.ActivationFunctionType
ALU = mybir.AluOpType
F32 = mybir.dt.float32
BF16 = mybir.dt.bfloat16

N_CORES = 8
BATCH, IN_DIM, UNITS = 65536, 256, 512
B_CORE = BATCH // N_CORES          # 8192 batch rows per core
C = 1024                           # batch columns per chunk
N_CHUNKS = B_CORE // C
NMM = 512                          # moving free dim per matmul (one PSUM bank)
N_SLICES = C // NMM
MU = UNITS // 128                  # 4 m-tiles per layer
KX = IN_DIM // 128                 # 2 k-tiles for x-side matmuls
KU = UNITS // 128                  # 4 k-tiles for unit-side matmuls

USE_CUSTOM_DVE = True              # v2c: fused (c0 - c1|z|)*sin DVE op
# skip tiles whose PSUM s is drained via ScalarE copy (index m in 0..3);
# the rest are added straight from PSUM on the DVE at 1x.
SCALAR_DRAIN_M = (0, 1, 2, 3)

_CACHE = {}


def _fit_exp_poly(d, umax, deg):
    """Near-minimax polynomial coefficients for exp(-d*u) on [0, umax]."""
    xs = np.cos(np.pi * (np.arange(512) + 0.5) / 512) * umax / 2 + umax / 2
    ch = np.polynomial.chebyshev.Chebyshev.fit(xs, np.exp(-d * xs), deg,
                                               domain=[0.0, umax])
    return ch.convert(kind=np.polynomial.Polynomial).coef


def _get_custom_op():
    """Register (once) the fused DVE op out = (s0 - |in0|*s1) * in1."""
    import concourse.dve_ops as dops
    from concourse.dve_spec import (Spec, Src0, Src1, C0, C1, Zero, maxx,
                                    lower, _has_src1)
    from concourse.dve_uop import DveOpSpec

    name = "SINE_DECAY_ANT"
    for o in dops.OPS:
        if o.name == name:
            return o
    az = maxx(Src0, Zero - Src0)
    spec = Spec(
        body=(C0 - az * C1) * Src1,
        reference=lambda in0, in1, s0, s1, imm2:
            ((s0 - np.abs(in0.astype(np.float32)) * s1)
             * in1.astype(np.float32)).astype(np.float32),
    )
    placeholder = dops.DveOp(name, spec, subdim=False, uops_sha={})
    dops.OPS.append(placeholder)
    dops._SUB_OPCODE_FOR_NAME[name] = dops._CUSTOM_DVE_ROW_BASE + len(dops.OPS) - 1
    dops.CUSTOM_DVE_SPECS[name] = spec
    shas = {}
    for ver in ("v3", "v4"):
        tmp = DveOpSpec(name=name, opcode=dops.get_dve_sub_opcode(name),
                        uops=lower(spec, ver=ver), rd1_en=_has_src1(spec))
        shas[ver] = tmp.sha(ver)
    op = dops.DveOp(name, spec, subdim=False, uops_sha=shas)
    dops.OPS[-1] = op
    dops.CUSTOM_DVE_SPECS[name] = spec
    return op


def _build(layer_params, zero_bias):
    """layer_params: list of 3 dicts with keys f, a, d, umax."""
    nc = bacc.Bacc("TRN2")
    custom_op = _get_custom_op() if (USE_CUSTOM_DVE and zero_bias) else None

    xT = nc.dram_tensor("xT", [IN_DIM, B_CORE], BF16, kind="ExternalInput")
    w0 = nc.dram_tensor("w0", [IN_DIM, UNITS], BF16, kind="ExternalInput")
    w1 = nc.dram_tensor("w1", [UNITS, UNITS], BF16, kind="ExternalInput")
    w2 = nc.dram_tensor("w2", [UNITS, UNITS], BF16, kind="ExternalInput")
    s1 = nc.dram_tensor("s1", [IN_DIM, UNITS], BF16, kind="ExternalInput")
    s2 = nc.dram_tensor("s2", [IN_DIM, UNITS], BF16, kind="ExternalInput")
    if not zero_bias:
        fb = [nc.dram_tensor(f"fb{l}", [UNITS, 1], F32, kind="ExternalInput")
              for l in range(3)]
        ab = [nc.dram_tensor(f"ab{l}", [UNITS, 1], F32, kind="ExternalInput")
              for l in range(3)]
    outT = nc.dram_tensor("outT", [3 * UNITS, B_CORE], BF16,
                          kind="ExternalOutput")

    # deg-1 fit of exp(-d u) on [0, umax], folded with amplitude a:
    # E = c0 + c1 u  (c1 < 0)
    pcoef = []
    for lp in layer_params:
        c = _fit_exp_poly(lp["d"], lp["umax"], 1) * lp["a"]
        assert c[1] < 0.0
        pcoef.append(list(c))

    with TileContext(nc) as tc:
        with (
            tc.tile_pool(name="wpool", bufs=1) as wpool,
            tc.tile_pool(name="xpool", bufs=3) as xpool,
            tc.tile_pool(name="hpool", bufs=4) as hpool,
            tc.tile_pool(name="opool", bufs=3) as opool,
            tc.tile_pool(name="ewpool", bufs=4) as ewpool,
            tc.tile_pool(name="zpool", bufs=5, space="PSUM") as zpool,
            tc.tile_pool(name="spool", bufs=3, space="PSUM") as spool,
        ):
            # ---- preload weights & biases ----
            # Critical-path loads (w0, x0, x1) go on the Sync engine so they
            # are not queued behind the 12 other weight tiles; the PE warms
            # its HAM clock gate on scratch matmuls in the meantime.
            def load_w(dram, kt, tag, eng):
                tiles = []
                for k in range(kt):
                    t = wpool.tile([128, UNITS], BF16, tag=f"{tag}_{k}",
                                   name=f"{tag}_{k}")
                    eng.dma_start(out=t, in_=dram[k * 128:(k + 1) * 128, :])
                    tiles.append(t)
                return tiles

            w0_t = load_w(w0, KX, "w0", nc.sync)

            # PE warmup: ~24 throwaway matmuls on a zeroed scratch tile keep
            # the HAM activity window busy while the first DMAs land, so real
            # matmuls start at 2.4 GHz instead of 1.2 GHz.
            WARMUP_MM = 0
            if WARMUP_MM:
                sw = wpool.tile([128, 128], BF16, tag="scratchw",
                                name="scratchw")
                sx = wpool.tile([128, NMM], BF16, tag="scratchx",
                                name="scratchx")
                nc.vector.memset(sw, 0)
                nc.vector.memset(sx, 0)
                for wi in range(WARMUP_MM):
                    zw = zpool.tile([128, C], F32, tag="z", name=f"warm_{wi}")
                    nc.tensor.matmul(zw[:, :NMM], sw, sx,
                                     start=True, stop=True)

            w_t = [w0_t, load_w(w1, KU, "w1", nc.gpsimd),
                   load_w(w2, KU, "w2", nc.gpsimd)]
            sk_t = [None, load_w(s1, KX, "s1", nc.gpsimd),
                    load_w(s2, KX, "s2", nc.gpsimd)]
            fb_t = [[0.0] * MU for _ in range(3)]
            ab_t = [[0.0] * MU for _ in range(3)]
            if not zero_bias:
                for l in range(3):
                    for m in range(MU):
                        tf = wpool.tile([128, 1], F32, tag=f"fb{l}_{m}",
                                        name=f"fb{l}_{m}")
                        nc.gpsimd.dma_start(
                            out=tf, in_=fb[l][m * 128:(m + 1) * 128, :])
                        ta = wpool.tile([128, 1], F32, tag=f"ab{l}_{m}",
                                        name=f"ab{l}_{m}")
                        nc.gpsimd.dma_start(
                            out=ta, in_=ab[l][m * 128:(m + 1) * 128, :])
                        fb_t[l][m] = tf
                        ab_t[l][m] = ta

            x_tiles = {}      # chunk -> list of KX tiles
            h_tiles = {}      # (chunk, layer) -> list of MU tiles

            def load_x(ci):
                """x tiles split per (k, n) so the first matmul of a chunk
                depends on a 128 KiB transfer, not 256 KiB."""
                if ci >= N_CHUNKS or ci in x_tiles:
                    return
                c0 = ci * C
                eng = nc.scalar if ci < 2 else nc.gpsimd
                ts = [[None] * N_SLICES for _ in range(KX)]
                for n in range(N_SLICES):
                    for k in range(KX):
                        xt = xpool.tile([128, NMM], BF16, tag=f"x{k}_{n}",
                                        name=f"x_{ci}_{k}_{n}")
                        eng.dma_start(
                            out=xt,
                            in_=xT[k * 128:(k + 1) * 128,
                                   c0 + n * NMM:c0 + (n + 1) * NMM])
                        ts[k][n] = xt
                x_tiles[ci] = ts

            def emit_layer(ci, l):
                if ci >= N_CHUNKS:
                    return
                c0 = ci * C
                lp = layer_params[l]
                co = pcoef[l]
                k_tiles = KX if l == 0 else KU
                h_prev = x_tiles[ci] if l == 0 else h_tiles[(ci, l - 1)]
                x_t = x_tiles[ci]
                h_cur = []
                last = sk_t[l] is None
                for m in range(MU):
                    h = (hpool.tile([128, C], BF16, tag=f"h{m}",
                                    name=f"h_{ci}_{l}_{m}")
                         if l < 2 else
                         opool.tile([128, C], BF16, tag="o",
                                    name=f"h_{ci}_{l}_{m}"))
                    for n in range(N_SLICES):
                        nsl = slice(n * NMM, (n + 1) * NMM)
                        z = zpool.tile([128, NMM], F32, tag="z",
                                       name=f"z_{ci}_{l}_{m}_{n}")
                        for k in range(k_tiles):
                            nc.tensor.matmul(
                                z,
                                w_t[l][k][:, m * 128:(m + 1) * 128],
                                x_t[k][n] if l == 0 else h_prev[k][:, nsl],
                                start=(k == 0), stop=(k == k_tiles - 1))
                        if not last:
                            s = spool.tile([128, NMM], F32, tag="s",
                                           name=f"s_{ci}_{l}_{m}_{n}")
                            for k in range(KX):
                                nc.tensor.matmul(
                                    s,
                                    sk_t[l][k][:, m * 128:(m + 1) * 128],
                                    x_t[k][n],
                                    start=(k == 0), stop=(k == KX - 1))

                        # sine_part = (c0 + c1|z + b|) * sin(f(z + b))
                        sin_t = ewpool.tile([128, NMM], BF16, tag="sin",
                                            name=f"sin_{ci}_{l}_{m}_{n}")
                        nc.scalar.activation(sin_t, z, AF.Sin,
                                             bias=fb_t[l][m], scale=lp["f"])

                        hs = h[:, nsl]
                        v = hs if last else ewpool.tile(
                            [128, NMM], BF16, tag="v",
                            name=f"v_{ci}_{l}_{m}_{n}")
                        if custom_op is not None:
                            # one DVE op drains z: (c0 - |z|*|c1|) * sin
                            nc.vector._custom_dve(custom_op, out=v, in0=z,
                                                  in1=sin_t, s0=co[0],
                                                  s1=-co[1])
                        else:
                            u_t = ewpool.tile([128, NMM], BF16, tag="u",
                                              name=f"u_{ci}_{l}_{m}_{n}")
                            nc.scalar.activation(u_t, z, AF.Abs,
                                                 bias=ab_t[l][m], scale=co[1])
                            e_t = ewpool.tile([128, NMM], BF16, tag="e",
                                              name=f"e_{ci}_{l}_{m}_{n}")
                            nc.vector.tensor_scalar(e_t, u_t, -1.0, co[0],
                                                    ALU.mult, ALU.add)
                            nc.vector.tensor_tensor(v, e_t, sin_t, ALU.mult)

                        if not last:
                            if m in SCALAR_DRAIN_M and not (m == 3 and n == 1):
                                s_sb = ewpool.tile([128, NMM], BF16, tag="ssb",
                                                   name=f"ssb_{ci}_{l}_{m}_{n}")
                                nc.scalar.copy(s_sb, s)
                                nc.vector.tensor_tensor(hs, v, s_sb, ALU.add)
                            else:
                                nc.vector.tensor_tensor(hs, v, s, ALU.add)

                        nc.sync.dma_start(
                            out=outT[l * UNITS + m * 128:
                                     l * UNITS + (m + 1) * 128,
                                     c0 + n * NMM:c0 + (n + 1) * NMM],
                            in_=hs)
                    h_cur.append(h)
                h_tiles[(ci, l)] = h_cur

            # ---- software-pipelined emission ----
            load_x(0)
            load_x(1)
            emit_layer(0, 0)
            for ci in range(N_CHUNKS):
                load_x(ci + 2)
                emit_layer(ci, 1)
                emit_layer(ci + 1, 0)
                emit_layer(ci, 2)
                # release dead references
                h_tiles.pop((ci, 0), None)
                h_tiles.pop((ci, 1), None)
                x_tiles.pop(ci, None)

    nc.finalize()
    return nc


def _bf16(a):
    return np.asarray(a, dtype=np.float32).astype(ml_dtypes.bfloat16)


def kernel(x, W0, b0, M0, f0, a0, d0,
           W1, b1, M1, f1, a1, d1, S1, SM1,
           W2, b2, M2, f2, a2, d2, S2, SM2,
           _trace=False):
    x = np.asarray(x, dtype=np.float32)
    W0m = _bf16(np.asarray(W0) * np.asarray(M0))
    W1m = _bf16(np.asarray(W1) * np.asarray(M1))
    W2m = _bf16(np.asarray(W2) * np.asarray(M2))
    S1m = _bf16(np.asarray(S1) * np.asarray(SM1))
    S2m = _bf16(np.asarray(S2) * np.asarray(SM2))
    fs = [float(f0), float(f1), float(f2)]
    as_ = [float(a0), float(a1), float(a2)]
    ds = [float(d0), float(d1), float(d2)]
    bs = [np.asarray(b0, dtype=np.float32).reshape(UNITS, 1),
          np.asarray(b1, dtype=np.float32).reshape(UNITS, 1),
          np.asarray(b2, dtype=np.float32).reshape(UNITS, 1)]
    zero_bias = all(not b.any() for b in bs)

    layer_params = [
        {"f": fs[0], "a": as_[0], "d": ds[0], "umax": 2.0},
        {"f": fs[1], "a": as_[1], "d": ds[1], "umax": 1.0},
        {"f": fs[2], "a": as_[2], "d": ds[2], "umax": 1.0},
    ]

    key = (zero_bias, USE_CUSTOM_DVE, SCALAR_DRAIN_M,
           tuple((lp["f"], lp["a"], lp["d"]) for lp in layer_params))
    if _CACHE.get("key") != key:
        _CACHE["nc"] = _build(layer_params, zero_bias)
        _CACHE["key"] = key
    nc = _CACHE["nc"]

    xT_full = np.ascontiguousarray(x.T).astype(ml_dtypes.bfloat16)  # [256, B]
    in_maps = []
    for c in range(N_CORES):
        m = {
            "xT": np.ascontiguousarray(xT_full[:, c * B_CORE:(c + 1) * B_CORE]),
            "w0": W0m, "w1": W1m, "w2": W2m, "s1": S1m, "s2": S2m,
        }
        if not zero_bias:
            for l in range(3):
                lp = layer_params[l]
                c1 = float((_fit_exp_poly(lp["d"], lp["umax"], 1) * lp["a"])[1])
                m[f"fb{l}"] = (fs[l] * bs[l]).astype(np.float32)
                m[f"ab{l}"] = (c1 * bs[l]).astype(np.float32)
        in_maps.append(m)

    res = bass_utils.run_bass_kernel_spmd(
        nc, in_maps, core_ids=list(range(N_CORES)), trace=_trace)

    out = np.empty((BATCH, 3 * UNITS), dtype=np.float32)
    for c in range(N_CORES):
        out[c * B_CORE:(c + 1) * B_CORE, :] = \
            res.results[c]["outT"].T.astype(np.float32)
    if _trace:
        _CACHE["last_result"] = res
    return out
